# revision 17
# baseline (speedup 1.0000x reference)
"""Single-head attention (B=8, S=4096, E=512, H=64) on 8 trn2 NeuronCores.

Sharding: data-parallel over batch - one batch element per core.

v4 design (ACT(exp)-roof ~150us, everything else hidden under it):
  - Host marshaling: x[b].T cast to bf16; mask transposed+inverted to
    m01T = (mask.T == 0) in bf16 {0,1}; weights bf16 with Wq/Wk duplicated
    column-wise ([Wq|Wq]) so the Q/K projections produce a vertically
    duplicated [128, S] layout for free (same matmul stream, M=128).
  - Scores computed TRANSPOSED and ROW-TILED: chunk pair (k1,k2) runs as two
    concurrent K=64 matmuls on row-groups 0-63 / 64-127 (tile_position),
    halving PE score time.
  - Mask applied two ways (split NPE_PAIRS, balances PE vs DVE):
      PE pairs:  sc += 32768 * m01T via stationary posI (= diag(+32768)
                 stacked twice), streamed m01T as rhs; exp bias -32768*scale
                 restores unmasked scores and underflows masked ones to 0.
      DVE pairs: at = exp(scale*sc) then at *= m01T (bf16 tensor_tensor 2x).
  - exp on ACT from PSUM at FD=1024 (one instruction per chunk pair), bf16
    out; attn@V per chunk in bf16, softmax denominator free via the ones
    column of V' (M=65).
  - Software pipelining: attn@V for pair g is emitted one pair late (and
    across q-block boundaries) so the exp latency never stalls the in-order
    PE queue; the output fixup for q-block qb is likewise deferred into
    qb+1's pair stream. Phase A (QKV) blocks are interleaved with the first
    q-block's pairs. All of this keeps the PE from idling >3.4us, which
    would drop its HAM clock from 2.4GHz to 1.2GHz.
  - x DMA and output ride the GpSimd queue, mask stripes ride Sync - the
    Scalar queue carries only exp (it is the bottleneck).
"""
import sys

sys.path.insert(0, "/opt/trn_rl_repo")

import ml_dtypes
import numpy as np

import concourse.bacc as bacc
import concourse.tile as tile
from concourse import mybir
from concourse.bass_utils import run_bass_kernel_spmd

F32 = mybir.dt.float32
BF16 = mybir.dt.bfloat16
NPBF16 = ml_dtypes.bfloat16

B, S, E, H = 8, 4096, 512, 64
SCALE = float(E) ** -0.5
POS = 32768.0
EXP_BIAS = -float(np.float32(POS) * np.float32(SCALE))

NPE_PAIRS = 0   # pairs per q-block masked on PE (rest: DVE multiply)


def build_program(s=S, npe=NPE_PAIRS):
    nc = bacc.Bacc("TRN2", target_bir_lowering=False, debug=False, num_devices=B)
    xT = nc.dram_tensor("xT", [E, s], BF16, kind="ExternalInput")
    m01T = nc.dram_tensor("m01T", [s, s], BF16, kind="ExternalInput")
    wq2 = nc.dram_tensor("wq2", [E, 128], BF16, kind="ExternalInput")
    wk2 = nc.dram_tensor("wk2", [E, 128], BF16, kind="ExternalInput")
    wv = nc.dram_tensor("wv", [E, H], BF16, kind="ExternalInput")
    b2q = nc.dram_tensor("b2q", [128, 1], F32, kind="ExternalInput")
    b2k = nc.dram_tensor("b2k", [128, 1], F32, kind="ExternalInput")
    bvb = nc.dram_tensor("bvb", [128, H], BF16, kind="ExternalInput")
    out = nc.dram_tensor("out", [s, H], F32, kind="ExternalOutput")

    NE = E // 128          # 4 E-chunks
    NB = s // 512          # 8 blocks of 512 (both s and q blocking)
    NQ = s // 128          # 32 key chunks of 128
    NP = NQ // 2           # 16 chunk pairs

    with tile.TileContext(nc) as tc:
        with (
            tc.tile_pool(name="const", bufs=1) as cst,
            tc.tile_pool(name="xp", bufs=3) as xp,
            tc.tile_pool(name="qkv", bufs=1) as qkv,
            tc.tile_pool(name="mstr", bufs=12) as mstr,
            tc.tile_pool(name="atp", bufs=6) as atp,
            tc.tile_pool(name="osb", bufs=2) as osb,
            tc.tile_pool(name="psS", bufs=2, space="PSUM") as psS,
            tc.tile_pool(name="psQK", bufs=2, space="PSUM") as psQK,
            tc.tile_pool(name="psOT", bufs=2, space="PSUM") as psOT,
        ):
            # ---- constants ----
            posI = cst.tile([128, 64], BF16)
            nc.gpsimd.memset(posI, 0.0)
            nc.gpsimd.affine_select(
                out=posI, in_=posI, compare_op=mybir.AluOpType.not_equal,
                fill=POS, base=0, pattern=[[-1, 64]], channel_multiplier=1,
            )
            nc.gpsimd.affine_select(
                out=posI, in_=posI, compare_op=mybir.AluOpType.not_equal,
                fill=POS, base=-64, pattern=[[-1, 64]], channel_multiplier=1,
            )
            idb = cst.tile([128, 128], BF16)
            nc.gpsimd.memset(idb, 0.0)
            nc.gpsimd.affine_select(
                out=idb, in_=idb, compare_op=mybir.AluOpType.not_equal,
                fill=1.0, base=0, pattern=[[-1, 128]], channel_multiplier=1,
            )
            ones1 = cst.tile([1, 128], F32)
            nc.vector.memset(ones1, 1.0)
            ebias = cst.tile([128, 1], F32)
            nc.vector.memset(ebias, EXP_BIAS)
            zbias = cst.tile([128, 1], F32)
            nc.vector.memset(zbias, 0.0)

            # x block 0 DMA first on the gpsimd queue: it gates phase A
            xts = [xp.tile([128, NE, 512], BF16, tag="xt", name=f"xt_{i}")
                   for i in range(NB)]
            nc.gpsimd.dma_start(
                out=xts[0],
                in_=xT[:, 0:512].rearrange("(c p) s -> p c s", p=128))

            wq2_sb = cst.tile([128, NE, 128], BF16)
            wk2_sb = cst.tile([128, NE, 128], BF16)
            wv_sb = cst.tile([128, NE, H], BF16)
            nc.sync.dma_start(
                out=wq2_sb, in_=wq2.rearrange("(c p) m -> p c m", p=128))
            nc.sync.dma_start(
                out=wk2_sb, in_=wk2.rearrange("(c p) m -> p c m", p=128))
            nc.sync.dma_start(
                out=wv_sb, in_=wv.rearrange("(c p) h -> p c h", p=128))
            bvb_sb = cst.tile([128, H], BF16)
            nc.sync.dma_start(out=bvb_sb, in_=bvb[:])
            b2q_sb = cst.tile([128, 1], F32)
            b2k_sb = cst.tile([128, 1], F32)
            nc.sync.dma_start(out=b2q_sb, in_=b2q[:])
            nc.sync.dma_start(out=b2k_sb, in_=b2k[:])

            # ---- mask stripe DMA (4x 1MiB sub-stripes per q-block, so the
            # first score pair is not gated on a full 4MiB transfer); these
            # lead the sync queue so sub-stripe (0,0) lands earliest ----
            stripes = []
            for qb in range(NB):
                subs = []
                for ss in range(4):
                    st = mstr.tile([128, 8, 512], BF16, tag="st",
                                   name=f"st_{qb}_{ss}")
                    nc.sync.dma_start(
                        out=st,
                        in_=m01T[ss * 1024:(ss + 1) * 1024,
                                 qb * 512:(qb + 1) * 512]
                        .rearrange("(c p) q -> p c q", p=128),
                    )
                    subs.append(st)
                stripes.append(subs)

            Q2 = [qkv.tile([128, 512], BF16, name=f"q2_{i}") for i in range(NB)]
            K2 = [qkv.tile([128, 512], BF16, name=f"k2_{i}") for i in range(NB)]
            # V' padded 65 -> 128 cols so attn@V LDWEIGHTS gets FWL (2x);
            # col H is the ones column (softmax denominator), cols H+1.. are 0
            VP = [qkv.tile([128, 128], BF16, name=f"vp_{i}") for i in range(NQ)]
            for k in range(NQ):
                nc.vector.memset(VP[k], 0.0)
                nc.vector.memset(VP[k][:, H:H + 1], 1.0)

            def emit_block_a(sb):
                """Phase A for one 512-col block: Q2/K2/V' chunks."""
                s0 = sb * 512
                xt = xts[sb]
                if sb > 0:
                    nc.gpsimd.dma_start(
                        out=xt,
                        in_=xT[:, s0:s0 + 512]
                        .rearrange("(c p) s -> p c s", p=128),
                    )
                q_ps = psQK.tile([128, 512], F32, tag="qk", name=f"q_ps_{sb}")
                k_ps = psQK.tile([128, 512], F32, tag="qk", name=f"k_ps_{sb}")
                for e in range(NE):
                    nc.tensor.matmul(q_ps, wq2_sb[:, e, :], xt[:, e, :],
                                     start=(e == 0), stop=(e == NE - 1))
                    nc.tensor.matmul(k_ps, wk2_sb[:, e, :], xt[:, e, :],
                                     start=(e == 0), stop=(e == NE - 1))
                nc.vector.tensor_scalar_add(Q2[sb], q_ps, b2q_sb)
                nc.vector.tensor_scalar_add(K2[sb], k_ps, b2k_sb)
                for j in range(4):
                    c0 = j * 128
                    v_ps = psQK.tile([128, H], F32, tag="qk",
                                     name=f"v_ps_{sb}_{j}")
                    for e in range(NE):
                        nc.tensor.matmul(
                            v_ps, xt[:, e, c0:c0 + 128], wv_sb[:, e, :],
                            start=(e == 0), stop=(e == NE - 1),
                        )
                    nc.vector.tensor_add(VP[sb * 4 + j][:, 0:H], v_ps, bvb_sb)

            # deferred work queues: attn@V one pair late, fixup one q-block late
            pend = []
            fixq = []

            def emit_attnv(flush=False):
                while len(pend) > (0 if flush else 1):
                    g, at, ot_ps = pend.pop(0)
                    k1, k2 = 2 * g, 2 * g + 1
                    nc.tensor.matmul(ot_ps, VP[k1], at[:, 0, :],
                                     start=(g == 0), stop=False)
                    nc.tensor.matmul(ot_ps, VP[k2], at[:, 1, :],
                                     start=False, stop=(g == NP - 1))

            def alloc_ot(qb):
                ots[qb] = psOT.tile([128, 512], F32, tag="ot", name=f"ot_{qb}")

            def emit_pair(qb, g, ot_ps):
                k1, k2 = 2 * g, 2 * g + 1
                pe_mask = (g < npe) or (qb == NB - 1 and g == NP - 1)
                st = stripes[qb][k1 // 8]
                kl1, kl2 = k1 % 8, k2 % 8
                sc = psS.tile([128, 1024], F32, tag="sc")
                nc.tensor.matmul(
                    sc[:, 0:512],
                    K2[k1 // 4][0:64, (k1 % 4) * 128:(k1 % 4 + 1) * 128],
                    Q2[qb][0:64, :],
                    start=True, stop=not pe_mask, tile_position=(0, 0),
                )
                nc.tensor.matmul(
                    sc[:, 512:1024],
                    K2[k2 // 4][64:128, (k2 % 4) * 128:(k2 % 4 + 1) * 128],
                    Q2[qb][64:128, :],
                    start=True, stop=not pe_mask, tile_position=(64, 0),
                )
                if pe_mask:
                    for h2, k in ((0, kl1), (1, kl2)):
                        c = 512 * h2
                        nc.tensor.matmul(
                            sc[0:64, c:c + 512], posI[0:64, :], st[0:64, k, :],
                            start=False, stop=True, tile_position=(0, 0),
                        )
                        nc.tensor.matmul(
                            sc[64:128, c:c + 512], posI[64:128, :],
                            st[64:128, k, :],
                            start=False, stop=True, tile_position=(64, 64),
                        )
                emit_attnv()  # previous pair's attn@V goes here in PE order
                at = atp.tile([128, 2, 512], BF16, tag="at")
                nc.scalar.activation(
                    at, sc, mybir.ActivationFunctionType.Exp,
                    scale=SCALE, bias=ebias if pe_mask else zbias,
                )
                if not pe_mask:
                    nc.vector.tensor_mul(at, at, st[:, kl1:kl1 + 2, :])
                pend.append((g, at, ot_ps))

            def emit_fixup(flush=False):
                while len(fixq) > (0 if flush else 1):
                    qb, ot_ps = fixq.pop(0)
                    q0 = qb * 512
                    oT = osb.tile([H + 1, 512], BF16, tag="oT")
                    nc.vector.tensor_copy(oT, ot_ps[0:H + 1, :])
                    of = osb.tile([128, 4, H], F32, tag="of")
                    for j in range(4):
                        fx = psQK.tile([128, H + 1], BF16, tag="qk",
                                       name=f"fx_{qb}_{j}")
                        nc.tensor.transpose(
                            fx, oT[:, 128 * j:128 * (j + 1)],
                            idb[0:H + 1, 0:H + 1],
                        )
                        rc = osb.tile([128, 1], F32, tag="rc")
                        nc.vector.reciprocal(rc, fx[:, H:H + 1])
                        nc.vector.tensor_scalar_mul(of[:, j, :], fx[:, 0:H], rc)
                        if qb == NB - 1:
                            nc.sync.dma_start(
                                out=out[q0 + 128 * j:q0 + 128 * (j + 1), :],
                                in_=of[:, j, :],
                            )
                    if qb == NB - 1:
                        pass  # per-j DMAs below
                    else:
                        nc.gpsimd.dma_start(
                            out=out[q0:q0 + 512, :]
                            .rearrange("(j p) h -> p j h", p=128),
                            in_=of,
                        )

            # ---- fused phase A + first two q-blocks, then the rest ----
            ots = {}
            emit_block_a(0)
            alloc_ot(0)
            alloc_ot(1)
            done1 = 0
            for sb in range(1, NB):
                emit_block_a(sb)
                emit_pair(0, 2 * (sb - 1), ots[0])
                emit_pair(0, 2 * (sb - 1) + 1, ots[0])
                if sb >= 2:
                    emit_pair(1, done1, ots[1])
                    done1 += 1
            for g in range(2 * (NB - 1), NP):
                emit_pair(0, g, ots[0])
            fixq.append((0, ots[0]))
            for g in range(done1, NP):
                emit_pair(1, g, ots[1])
                if g == done1 + 3:
                    emit_fixup(flush=True)

            for qb in range(2, NB):
                alloc_ot(qb)
                fixq.append((qb - 1, ots[qb - 1]))
                for g in range(NP):
                    emit_pair(qb, g, ots[qb])
                    if g == 3:
                        # previous q-block's fixup: its deps completed early
                        # in this q-block, so the PE never waits on it here
                        emit_fixup(flush=True)
            fixq.append((NB - 1, ots[NB - 1]))
            emit_attnv(flush=True)
            emit_fixup(flush=True)
    nc.compile()
    return nc


def make_in_maps(x, attention_mask, Wq, bq, Wk, bk, Wv, bv):
    nb = x.shape[0]
    wq2 = np.concatenate([Wq, Wq], axis=1).astype(NPBF16)
    wk2 = np.concatenate([Wk, Wk], axis=1).astype(NPBF16)
    b2q = np.concatenate([bq, bq]).reshape(128, 1).astype(np.float32)
    b2k = np.concatenate([bk, bk]).reshape(128, 1).astype(np.float32)
    bvb = np.tile(np.asarray(bv, np.float32).reshape(1, H), (128, 1))
    common = {
        "wq2": wq2, "wk2": wk2, "wv": np.asarray(Wv).astype(NPBF16),
        "b2q": b2q, "b2k": b2k, "bvb": bvb.astype(NPBF16),
    }
    return [
        {
            "xT": np.ascontiguousarray(x[b].T).astype(NPBF16),
            "m01T": (attention_mask[b].T == 0).astype(NPBF16),
            **common,
        }
        for b in range(nb)
    ]


_PROGRAM = None


def kernel(x, attention_mask, Wq, bq, Wk, bk, Wv, bv):
    global _PROGRAM
    x = np.asarray(x, np.float32)
    attention_mask = np.asarray(attention_mask, np.int32)
    if _PROGRAM is None:
        _PROGRAM = build_program()
    in_maps = make_in_maps(
        x, attention_mask,
        np.asarray(Wq, np.float32), np.asarray(bq, np.float32),
        np.asarray(Wk, np.float32), np.asarray(bk, np.float32),
        np.asarray(Wv, np.float32), np.asarray(bv, np.float32),
    )
    res = run_bass_kernel_spmd(_PROGRAM, in_maps, core_ids=list(range(B)))
    return np.stack([res.results[b]["out"] for b in range(B)], axis=0)
